# revision 29
# baseline (speedup 1.0000x reference)
"""Trainium2 Bass kernel for nn_DigitConvolutionalModel (dense_cnn).

Model: y = relu(conv3x3(x) @ w1.T + b1) @ w2.T + b2, x: [65536, 784] f32.

Strategy:
  * Conv3x3 and FC1 fuse on the host into one effective weight
    W1e = w1 @ C with shape [128, 784] (C is the sparse conv operator),
    so the device runs a pure GEMM pipeline:
    y = relu(x @ W1e.T + b1) @ w2.T + b2.
  * Pure data parallel over 8 NeuronCores: each core gets 8192 rows of
    x. No collectives; each core produces its own output shard.
  * x travels as fp8e3 (e3m4), scaled by 2 on the host with the inverse
    folded into W1e (fp16) - mixed-dtype PE operands, fp32 PSUM
    accumulation; x quantization costs ~1.26e-2 rel_fro (gate 2e-2).
  * The kernel is PE-roofline bound when warm (96 N=512 fp16-rate
    matmuls ~20.7 us) with the x stream (~6.4 MB fp8) just underneath;
    the optimization battle is the edges:
    - All x on the SP HWDGE ring in strict consumption order. The SDMA
      engines round-robin across every outstanding transfer, so any
      concurrent stream dilutes the first loads, which gate warm-up:
      block 0 arrives as two 3-chunk pieces (196 KB each), and the
      middle blocks ride 1 MB pair-loads whose 6 KB descriptors run
      near the full HBM rate. The ACT ring carries only the small
      weight/bias transfers.
    - 9 dummy pre-context matmuls bridge the PE from the framework
      preamble (~7 us) to the first x piece's consumability (~11 us:
      first HWDGE issue slot + descriptor generation + transfer +
      completion receipt) with NO idle gap - the HAM activity window
      only unthrottles the PE clock (1.2 -> 2.4 GHz) after ~3.4 us of
      CONTINUOUS busy, and any gap restarts it.
    - The packed weights split at column 640 (chunks 0-4 | chunk 5 +
      tail + w2) so block 0's last chunk is the only matmul gated on
      the second piece, which lands well before it.
    - Filler matmuls (already-resident operands -> scratch bank)
      pad the two early x-arrival stalls (block 0's second piece,
      block 1) so the PE never idles: nearly free when the stall is
      real, and they keep the HAM window accumulating.
    - The exit chain is minimized: groups [0-3],[4-7],[8-11],
      [12,13,14],[15]; block 15's FC1 runs as two half-banks (N=256)
      so its bias+relu runs on BOTH the vector and scalar engines in
      parallel (different PSUM banks); the final FC2 runs as two
      single-matmul half-bank chains (start=True needs no opener when
      nothing writes the bank concurrently; one half reuses the
      scratch bank to dodge pool recycling), each gating only on its
      own relu half, with two [32, 256] bias-adds and one 32 KB final
      store. All bulk stores issue from the otherwise-idle Sync queue
      so the ACT engine is clear for the final chain; fc2 of
      [12,13,14] evacuates through the vector engine for the same
      reason. The grouped FC2s keep their N=1 opener matmul: a
      start=True on one of several concurrent col-group matmuls races
      the bank clear against the others' writes (HW-verified wrong
      results).
  * The 16-feature contraction tail (features 768:784, whole batch) is
    packed [128, 2048] across 4 row-groups of 32 partitions; w1e's
    tail rows are replicated at partition offsets 0/32/64/96 so each
    block's tail matmul reads its group via tile_position (row-group =
    bi % 4). A group's tail matmuls issue back-to-back on distinct
    row-groups and run concurrently in the PE array.
  * Bias+ReLU (PSUM -> SBUF fp16) alternates between the vector and
    scalar engines so a group's relus don't serialize on one engine.
  * FC2 runs software-pipelined one group late: a tiny N=1 matmul with
    a zero moving operand opens the accumulation group, then the
    group's [10, 512] FC2 matmuls target col-groups j=bi%4 of that ONE
    bank (disjoint partitions, start=False) and run concurrently in
    the array; one bias-add per group evacuates the bank into a
    [128, 2048] fp16 output accumulator.
  * Cross-engine waits are absorbed into the PE stream with tiny dummy
    bf16 ldweights "probes" (only where the data provably arrives
    before the probe executes); remaining multi-waits are split via
    event semaphores (bass_rust.generate_event_semaphores).
"""

import ml_dtypes
import numpy as np

import concourse.bass as bass
import concourse.mybir as mybir
import concourse.tile as tile
from concourse.bass import ts
from concourse.bass_utils import run_bass_kernel_spmd

H = W = 28
KH = KW = 3
CIN = H * W  # 784
HID = 128
OUT = 10
B_TOTAL = 65536
NCORES = 8
BS = B_TOTAL // NCORES  # 8192 rows per core
NB = 512  # batch columns per block (fp32 PSUM bank limit)
NBLK = BS // NB  # 16
KCH = 128
KC = 6  # full chunks (6 * 128 = 768)
KTAIL = CIN - KC * KCH  # 16
NGRP = 4  # tail row-groups / FC2 col-groups (32 partitions each)
TGC = BS // NGRP  # tail columns per group (2048)
WSPLIT = 640  # wpk head/rest split (chunks 0-4 | chunk 5 + tail + w2)
NDUMMY = 9  # pre-context HAM warm-up matmuls

HOST_DT = np.float16
X_DT = ml_dtypes.float8_e3m4
X_SCALE = 2.0  # folded into W1e on the host

# processing groups: three quads, a triple, and a final single block so
# the last FC2->act->store chain is as shallow as possible (the
# block->col-group/row-group maps are position-independent: j = bi % 4,
# q = bi // 4)
PGROUPS = [[0, 1, 2, 3], [4, 5, 6, 7], [8, 9, 10, 11], [12, 13, 14], [15]]


def _build_nc():
    f32 = mybir.dt.float32
    mdt = mybir.dt.float16
    xdt = mybir.dt.float8e3
    nc = bass.Bass()
    # x, host-pretiled per load: xa/xm entries are each one contiguous
    # [128, 6, ncols] region (features 0:768); xa = blocks 0-5, 14, 15,
    # xm = block pairs (6,7) (8,9) (10,11) (12,13)
    xa = nc.dram_tensor("xa", [8, KCH, KC, NB], xdt, kind="ExternalInput")
    xm = nc.dram_tensor("xm", [4, KCH, KC, 2 * NB], xdt, kind="ExternalInput")
    # x contraction tail (features 768:784) packed into 4 row-groups:
    # partition 32g+j = tail feature j of blocks 4g..4g+3
    xtl = nc.dram_tensor("xtl", [KCH, TGC], xdt, kind="ExternalInput")
    # all fp16 weights packed into one tensor, loaded in two pieces:
    # cols 0:768 = w1e chunks [k, c, m]; rows 32g:32g+16 of cols 768:896
    # = the 16-row w1e tail (replicated per row-group g); cols 896:906 =
    # w2t; col 906 zero (FC2 group opener)
    # first-gate transfer: block 0's first 3 chunks (1536 fp8 bytes)
    # + the weight head (WSPLIT fp16 = 1280 bytes) in ONE region, so the
    # first matmul waits on a single DMA completion instead of the max
    # of two jittery arrivals on different rings
    wx0 = nc.dram_tensor("wx0", [KCH, 1536 + 2 * WSPLIT], xdt, kind="ExternalInput")
    wpkb = nc.dram_tensor("wpkb", [KCH, 908 - WSPLIT], mdt, kind="ExternalInput")
    # biases in one f32 tensor: col 0 = b1, col 1 rows 32j:32j+10 = b2
    # (replicated per FC2 col-group j)
    bd = nc.dram_tensor("bd", [HID, 2], f32, kind="ExternalInput")
    # output, fp16, col-group packed: yt[32*(bi%4)+r, (bi//4)*512+c] =
    # y[bi*512+c, r]
    yt = nc.dram_tensor("yt", [KCH, NGRP * NB], mdt, kind="ExternalOutput")

    from contextlib import ExitStack

    es = ExitStack()
    # Pre-TileContext HAM warm-up (see module docstring). They read
    # uninitialized manual SBUF (values irrelevant) and write a scratch
    # PSUM bank nobody reads.
    wsb = es.enter_context(nc.sbuf_tensor([KCH, NB], mdt))
    wps = es.enter_context(nc.psum_tensor([KCH, NB], f32))

    for _ in range(NDUMMY):
        nc.tensor.matmul(wps[:], wsb[:, 0:HID], wsb[:], start=True, stop=True)

    with tile.TileContext(nc) as tc:
        with (
            tc.tile_pool(name="consts", bufs=1) as consts,
            tc.tile_pool(name="xin", bufs=1) as xin,
            tc.tile_pool(name="hpool", bufs=NBLK) as hpool,
            tc.tile_pool(name="ps1", bufs=5, space="PSUM") as ps1p,
            tc.tile_pool(name="ps2", bufs=2, space="PSUM") as ps2p,
        ):
            xts = {}  # block -> (tile, col offset)

            def xload(engine, tag, srcap, shape):
                t = xin.tile(shape, xdt, tag=tag, bufs=1, name=tag)
                engine.dma_start(t[:], srcap)
                return t

            # All x on the SP ring in strict consumption order;
            # the combined first-gate piece goes first.
            wx0_t = xin.tile(
                [KCH, 1536 + 2 * WSPLIT], xdt, tag="wx0", bufs=1, name="wx0"
            )
            nc.sync.dma_start(wx0_t[:], wx0[:])
            x0a = wx0_t[:, 0:1536].rearrange("k (c m) -> k c m", c=3)
            wpka_t = wx0_t[:, 1536:].bitcast(mdt)
            x0b = xload(nc.sync, "x0b", xa[0][:, 3:6, :], [KCH, 3, NB])
            xts[1] = (xload(nc.sync, "x1", xa[1][:], [KCH, KC, NB]), 0)
            x_tl = consts.tile([KCH, TGC], xdt)
            nc.sync.dma_start(x_tl[:], xtl[:])
            for bi in range(2, 6):
                xts[bi] = (xload(nc.sync, f"x{bi}", xa[bi][:], [KCH, KC, NB]), 0)
            for i in range(4):
                t = xload(nc.sync, f"xm{i}", xm[i][:], [KCH, KC, 2 * NB])
                xts[6 + 2 * i], xts[7 + 2 * i] = (t, 0), (t, NB)
            xts[14] = (xload(nc.sync, "x14", xa[6][:], [KCH, KC, NB]), 0)
            xts[15] = (xload(nc.sync, "x15", xa[7][:], [KCH, KC, NB]), 0)

            # ACT ring: only the small weight/bias transfers.
            wpkb_t = consts.tile([KCH, 908 - WSPLIT], mdt)
            nc.scalar.dma_start(wpkb_t[:], wpkb[:])
            bd_t = consts.tile([HID, 2], f32)
            nc.scalar.dma_start(bd_t[:], bd[:])
            b1_t = bd_t[:, 0:1]

            def w1_chunk(c):
                if c < 5:
                    return wpka_t[:, ts(c, KCH)]
                return wpkb_t[:, ts(c - 5, KCH)]

            wtail = wpkb_t[:, 128:256]  # [128, 128]: tail rows per group
            w2_t = wpkb_t[:, 256:266]
            wzero = wpkb_t[:, 266:267]  # zero column (FC2 opener)

            # fp16 output accumulator [128, 2048]
            o_all = consts.tile([KCH, NGRP * NB], mdt)

            # Tiny dummy bf16 ldweights "probes" absorb cross-engine
            # waits into the PE's in-order stream ahead of each matmul
            # group (walrus: one sync wait per instruction; the loaded
            # garbage weight is irrelevant, real matmuls self-load).
            def probe(ap, cast=True):
                ap = ap[0:1, 0:1]
                if cast:
                    ap = ap.bitcast(mybir.dt.bfloat16)
                nc.tensor.ldweights(ap)

            # Pre-touch the bias tile on the engines that consume it.
            b1_probe = consts.tile([1, 1], f32)
            nc.vector.tensor_copy(b1_probe[:], b1_t[0:1, 0:1])
            b2_probe = consts.tile([1, 1], f32)
            nc.scalar.copy(b2_probe[:], bd_t[0:1, 1:2])

            # Only pre-touch the weight head on the PE: the first real
            # matmul gates on it anyway.
            probe(wpka_t[:, 0:1])

            hs = [None] * NBLK

            def filler(n=1):
                """Keep the PE (and its HAM activity window) busy
                across an x-arrival stall: matmuls on already-resident
                operands into the scratch bank. Nearly free when the
                stall is real (the DMA pacer is unaffected); ~216 ns
                each when it isn't."""
                for _ in range(n):
                    nc.tensor.matmul(
                        wps[:], wpka_t[:, 0:HID], x0a[:, 0, :],
                        start=True, stop=True,
                    )

            def fc1_block(bi, ps):
                """6 accumulating FC1 matmuls for one full block."""
                if bi == 0:
                    probe(x0a[:, 0, 0:1], cast=False)
                    for c in range(3):
                        nc.tensor.matmul(
                            ps[:], w1_chunk(c), x0a[:, c, :],
                            start=(c == 0), stop=False,
                        )
                    filler(2)
                    probe(x0b[:, 0, 0:1], cast=False)
                    for c in range(3, KC):
                        if c == 5:
                            probe(wpkb_t[:, 0:1])
                        nc.tensor.matmul(
                            ps[:], w1_chunk(c), x0b[:, c - 3, :],
                            start=False, stop=False,
                        )
                else:
                    x_t, off = xts[bi]
                    probe(x_t[:, 0, off : off + 1], cast=False)
                    for c in range(KC):
                        nc.tensor.matmul(
                            ps[:], w1_chunk(c), x_t[:, c, off : off + NB],
                            start=(c == 0), stop=False,
                        )

            def tail_mm(bi, ps, c0, ncols):
                j, q = bi % NGRP, bi // NGRP
                nc.tensor.matmul(
                    ps[:, 0:ncols],
                    wtail[32 * j : 32 * j + KTAIL, :],
                    x_tl[32 * j : 32 * j + KTAIL, q * NB + c0 : q * NB + c0 + ncols],
                    start=False,
                    stop=True,
                    tile_position=(32 * j, 0),
                )

            def fc2_batch(blocks, evac_vector=False):
                """FC2 for a group of blocks (software-pipelined one
                group late). A tiny N=1 matmul opens the accumulation
                group (clears has_written + pending-zero for the bank);
                the FC2 matmuls then target col-groups j=bi%4 of that
                one bank (disjoint partitions, start=False = overwrite-
                where-unwritten) and run concurrently in the array; one
                bias-add over the written partition range evacuates the
                group."""
                q = blocks[0] // NGRP
                ps2 = ps2p.tile([KCH, NB], f32, tag="ps2", bufs=2)
                nc.tensor.matmul(
                    ps2[:, 0:1], wtail, wzero, start=True, stop=False
                )
                for n, bi in enumerate(blocks):
                    j = bi % NGRP
                    nc.tensor.matmul(
                        ps2[32 * j : 32 * j + OUT, :],
                        w2_t[:],
                        hs[bi][:],
                        start=False,
                        stop=(n == len(blocks) - 1),
                        tile_position=(0, 32 * j),
                    )
                j0, j1 = blocks[0] % NGRP, blocks[-1] % NGRP
                lo, hi = 32 * j0, 32 * j1 + 32
                if evac_vector:
                    nc.vector.tensor_scalar(
                        o_all[lo:hi, ts(q, NB)],
                        ps2[lo:hi, :],
                        bd_t[lo:hi, 1:2],
                        None,
                        mybir.AluOpType.add,
                    )
                else:
                    nc.scalar.activation(
                        o_all[lo:hi, ts(q, NB)],
                        ps2[lo:hi, :],
                        mybir.ActivationFunctionType.Identity,
                        bias=bd_t[lo:hi, 1:2],
                    )

            def relu(bi, ps, h_ap, ncols, on_vector):
                """h = max(ps + b1, 0), PSUM fp32 -> SBUF fp16."""
                if on_vector:
                    nc.vector.tensor_scalar(
                        h_ap,
                        ps[:, 0:ncols],
                        b1_t[:],
                        0.0,
                        mybir.AluOpType.add,
                        mybir.AluOpType.max,
                    )
                else:
                    nc.scalar.activation(
                        h_ap,
                        ps[:, 0:ncols],
                        mybir.ActivationFunctionType.Relu,
                        bias=b1_t[:],
                    )

            # ---- main pipeline: groups 0..3 ----
            for qi in range(4):
                blocks = PGROUPS[qi]
                pss = []
                for bi in blocks:
                    if bi == 1:
                        filler(3)
                    elif bi == 2:
                        filler(1)
                    ps = ps1p.tile([HID, NB], f32, tag="ps")
                    fc1_block(bi, ps)
                    pss.append(ps)
                for n, bi in enumerate(blocks):
                    tail_mm(bi, pss[n], 0, NB)
                for n, bi in enumerate(blocks):
                    h = hpool.tile([HID, NB], mdt, tag="h", name=f"h_{bi}")
                    relu(bi, pss[n], h[:], NB, on_vector=bi not in (11, 13))
                    hs[bi] = h
                if qi >= 1:
                    fc2_batch(PGROUPS[qi - 1])
                if qi == 3:
                    # cols 0:1536 (q=0,1,2) complete after fc2_batch
                    # (PGROUPS[2]) just above: bulk store overlaps the
                    # rest of the compute (issued from the idle Sync
                    # queue so the ACT engine stays clear)
                    nc.sync.dma_start(yt[:, 0 : 3 * NB], o_all[:, 0 : 3 * NB])

            # ---- epilogue: block 15, two half-banks for a parallel
            # relu and the shallowest possible final chain ----
            psA = ps1p.tile([HID, NB], f32, tag="ps")
            psB = ps1p.tile([HID, NB], f32, tag="ps")
            x_t, off = xts[15]
            probe(x_t[:, 0, off : off + 1], cast=False)
            for c in range(KC):
                wc = w1_chunk(c)
                nc.tensor.matmul(
                    psA[:, 0:256], wc, x_t[:, c, off : off + 256],
                    start=(c == 0), stop=False,
                )
                nc.tensor.matmul(
                    psB[:, 0:256], wc, x_t[:, c, off + 256 : off + NB],
                    start=(c == 0), stop=False,
                )
            # block 15 tails + relus fire first (relu15b is the only
            # pre-final-act ACT work); FC2 of [12,13,14] fills the PE
            # while they resolve, its bias-add runs on DVE and its
            # store issues from the idle Sync queue - the ACT engine
            # goes straight from relu15b to the final bias-adds + store.
            tail_mm(15, psA, 0, 256)
            tail_mm(15, psB, 256, 256)
            h15 = hpool.tile([HID, NB], mdt, tag="h", name="h_15")
            relu(15, psA, h15[:, 0:256], 256, on_vector=True)
            relu(15, psB, h15[:, 256:NB], 256, on_vector=False)
            hs[15] = h15
            fc2_batch(PGROUPS[3], evac_vector=True)
            nc.sync.dma_start(yt[0:96, 3 * NB :], o_all[0:96, 3 * NB :])
            # final FC2 as two independent half-bank chains: each half
            # gates only on its own relu half, and the two [32, 256]
            # bias-adds are cheaper than one [32, 512] on the critical
            # path (different PSUM banks, so no read-write collision).
            # Half b uses the pre-context scratch bank (free since the
            # dummies) so neither half waits on ps2-pool recycling,
            # which would chain it behind [12,13,14]'s late evacuation.
            for half in range(2):
                c0 = 256 * half
                if half == 0:
                    ps2h = ps2p.tile([KCH, NB], f32, tag="ps2", bufs=2)
                    ps2 = ps2h[:]
                else:
                    ps2 = wps[:]
                nc.tensor.matmul(
                    ps2[96:106, 0:256],
                    w2_t[:],
                    h15[:, c0 : c0 + 256],
                    start=True,
                    stop=True,
                    tile_position=(0, 96),
                )
                nc.scalar.activation(
                    o_all[96:128, 3 * NB + c0 : 3 * NB + c0 + 256],
                    ps2[96:128, 0:256],
                    mybir.ActivationFunctionType.Identity,
                    bias=bd_t[96:128, 1:2],
                )

            # Final store (32 KB) from the ACT sequencer: program order
            # after the acts means it needs no cross-engine waits at all.
            nc.scalar.dma_start(yt[96:128, 3 * NB :], o_all[96:128, 3 * NB :])

    # This walrus build allows one sync-wait per instruction; Tile emits
    # multi-waits in a few places. Split them into event-semaphore
    # chains, same as bacc.compile() does.
    import bass_rust

    bass_rust.generate_event_semaphores(nc)
    es.close()
    return nc


def _fuse_conv_fc1(conv_w, w1):
    """W1e = w1 @ C where C is the 3x3 valid-conv operator [676, 784]."""
    cw = np.asarray(conv_w, np.float64).reshape(KH, KW)
    w1_r = np.asarray(w1, np.float64).reshape(HID, H - KH + 1, W - KW + 1)
    w1e = np.zeros((HID, H, W), np.float64)
    for a in range(KH):
        for b in range(KW):
            w1e[:, a : a + H - KH + 1, b : b + W - KW + 1] += w1_r * cw[a, b]
    return w1e.reshape(HID, CIN).astype(np.float32)


def _tile_cols(x_shard, cs, ncols):
    """[128, 6, ncols] contiguous device layout for columns cs:cs+ncols."""
    return (
        x_shard[cs : cs + ncols, : KC * KCH]
        .reshape(ncols, KC, KCH)
        .transpose(2, 1, 0)
        .astype(X_DT)
    )


def _core_x(x_shard):
    """Pre-tile one core's x rows [BS, 784] into the device layout.
    x arrives pre-scaled by X_SCALE."""
    xa = np.stack(
        [_tile_cols(x_shard, bi * NB, NB) for bi in range(6)]
        + [
            _tile_cols(x_shard, BS - 2 * NB, NB),
            _tile_cols(x_shard, BS - NB, NB),
        ]
    )
    xm = np.stack(
        [_tile_cols(x_shard, 6 * NB + 2 * NB * i, 2 * NB) for i in range(4)]
    )
    xtl = np.zeros((KCH, TGC), X_DT)
    tail = x_shard[:, KC * KCH :].astype(X_DT)  # [BS, 16]
    for bi in range(NBLK):
        q, j = divmod(bi, NGRP)
        xtl[32 * j : 32 * j + KTAIL, q * NB : (q + 1) * NB] = tail[
            bi * NB : (bi + 1) * NB
        ].T
    return (
        np.ascontiguousarray(xa),
        np.ascontiguousarray(xm),
        np.ascontiguousarray(xtl),
    )


def _host_weights(conv_w, w1, b1, w2, b2):
    """Pack all fp16 weights into wpk [128, 908] and biases into bd."""
    # 1/X_SCALE folds into W1e (exact in fp16: pure exponent shift)
    w1e_t = (_fuse_conv_fc1(conv_w, w1).T / X_SCALE).astype(HOST_DT)  # [784, 128]
    w2t = np.asarray(w2, np.float32).T.astype(HOST_DT)  # [128, 10]
    wpk = np.zeros((KCH, 908), HOST_DT)
    wpk[:, 0:768] = (
        w1e_t[0 : KC * KCH].reshape(KC, KCH, HID).transpose(1, 0, 2).reshape(KCH, -1)
    )
    for g in range(NGRP):
        wpk[32 * g : 32 * g + KTAIL, 768:896] = w1e_t[KC * KCH :]
    wpk[:, 896:906] = w2t
    bd = np.zeros((HID, 2), np.float32)
    bd[:, 0] = np.asarray(b1, np.float32)
    for j in range(NGRP):
        bd[32 * j : 32 * j + OUT, 1] = np.asarray(b2, np.float32)
    return (
        np.ascontiguousarray(wpk[:, :WSPLIT]),
        np.ascontiguousarray(wpk[:, WSPLIT:]),
        np.ascontiguousarray(bd),
    )


def _run(x, conv_w, w1, b1, w2, b2, trace=False):
    x = np.asarray(x, np.float32) * np.float32(X_SCALE)
    wpka, wpkb, bd = _host_weights(conv_w, w1, b1, w2, b2)
    wpka_bytes = np.ascontiguousarray(wpka).view(np.uint8)  # [128, 1024+256]

    nc = _build_nc()
    in_maps = []
    for c in range(NCORES):
        xa, xm, xtl = _core_x(x[c * BS : (c + 1) * BS])
        wx0 = np.ascontiguousarray(
            np.concatenate(
                [xa[0, :, 0:3, :].reshape(KCH, 3 * NB).view(np.uint8), wpka_bytes],
                axis=1,
            ).view(X_DT)
        )
        in_maps.append(
            {"xa": xa, "xm": xm, "xtl": xtl, "wx0": wx0, "wpkb": wpkb, "bd": bd}
        )
    res = run_bass_kernel_spmd(nc, in_maps, list(range(NCORES)), trace=trace)

    y = np.empty((B_TOTAL, OUT), np.float32)
    for c, r in enumerate(res.results):
        # yt[32j+r, 512q+cc] = y[(4q+j)*512+cc, r]
        ytc = np.asarray(r["yt"], np.float32).reshape(NGRP, 32, NGRP, NB)[:, :OUT]
        y[c * BS : (c + 1) * BS] = ytc.transpose(2, 0, 3, 1).reshape(BS, OUT)
    return y, res


def kernel(x, conv_w, w1, b1, w2, b2):
    y, _ = _run(x, conv_w, w1, b1, w2, b2)
    return y


# revision 30
# speedup vs baseline: 1.0220x; 1.0220x over previous
"""Trainium2 Bass kernel for nn_DigitConvolutionalModel (dense_cnn).

Model: y = relu(conv3x3(x) @ w1.T + b1) @ w2.T + b2, x: [65536, 784] f32.

Strategy:
  * Conv3x3 and FC1 fuse on the host into one effective weight
    W1e = w1 @ C with shape [128, 784] (C is the sparse conv operator),
    so the device runs a pure GEMM pipeline:
    y = relu(x @ W1e.T + b1) @ w2.T + b2.
  * Pure data parallel over 8 NeuronCores: each core gets 8192 rows of
    x. No collectives; each core produces its own output shard.
  * x travels as fp8e3 (e3m4), scaled by 2 on the host with the inverse
    folded into W1e (fp16) - mixed-dtype PE operands, fp32 PSUM
    accumulation; x quantization costs ~1.26e-2 rel_fro (gate 2e-2).
  * The kernel is PE-roofline bound when warm (96 N=512 fp16-rate
    matmuls ~20.7 us) with the x stream (~6.4 MB fp8) just underneath;
    the optimization battle is the edges:
    - All x on the SP HWDGE ring in strict consumption order. The SDMA
      engines round-robin across every outstanding transfer, so any
      concurrent stream dilutes the first loads, which gate warm-up:
      the first transfer fuses block 0's first 3 chunks WITH the fp16
      weight head into one 352 KB region (bitcast views on the SBUF
      tile), so the first matmul gates on a single DMA completion;
      block 0's rest follows as a 3-chunk piece, and the middle blocks
      ride 1 MB pair-loads whose 6 KB descriptors run near the full
      HBM rate. The ACT ring carries only the small remaining
      weight/bias transfers.
    - 9 dummy pre-context matmuls bridge the PE from the framework
      preamble (~7 us) to the first x piece's consumability (~11 us:
      first HWDGE issue slot + descriptor generation + transfer +
      completion receipt) with NO idle gap - the HAM activity window
      only unthrottles the PE clock (1.2 -> 2.4 GHz) after ~3.4 us of
      CONTINUOUS busy, and any gap restarts it.
    - The packed weights split at column 640 (chunks 0-4 | chunk 5 +
      tail + w2) so block 0's last chunk is the only matmul gated on
      the second piece, which lands well before it.
    - Filler matmuls (already-resident operands -> scratch bank)
      pad the two early x-arrival stalls (block 0's second piece,
      block 1) so the PE never idles: nearly free when the stall is
      real, and they keep the HAM window accumulating.
    - The exit chain is minimized: groups [0-3],[4-7],[8-11],
      [12,13,14],[15]; block 15's FC1 runs as two half-banks (N=256)
      so its bias+relu runs on BOTH the vector and scalar engines in
      parallel (different PSUM banks); the final FC2 runs as two
      single-matmul half-bank chains (start=True needs no opener when
      nothing writes the bank concurrently; one half reuses the
      scratch bank to dodge pool recycling), each gating only on its
      own relu half, with two [32, 256] bias-adds and one 32 KB final
      store. All bulk stores issue from the otherwise-idle Sync queue
      so the ACT engine is clear for the final chain; fc2 of
      [12,13,14] evacuates through the vector engine for the same
      reason. The grouped FC2s keep their N=1 opener matmul: a
      start=True on one of several concurrent col-group matmuls races
      the bank clear against the others' writes (HW-verified wrong
      results).
  * The 16-feature contraction tail (features 768:784, whole batch) is
    packed [128, 2048] across 4 row-groups of 32 partitions; w1e's
    tail rows are replicated at partition offsets 0/32/64/96 so each
    block's tail matmul reads its group via tile_position (row-group =
    bi % 4). A group's tail matmuls issue back-to-back on distinct
    row-groups and run concurrently in the PE array.
  * Bias+ReLU (PSUM -> SBUF fp16) alternates between the vector and
    scalar engines so a group's relus don't serialize on one engine.
  * FC2 runs software-pipelined one group late: a tiny N=1 matmul with
    a zero moving operand opens the accumulation group, then the
    group's [10, 512] FC2 matmuls target col-groups j=bi%4 of that ONE
    bank (disjoint partitions, start=False) and run concurrently in
    the array; one bias-add per group evacuates the bank into a
    [128, 2048] fp16 output accumulator.
  * Cross-engine waits are absorbed into the PE stream with tiny dummy
    bf16 ldweights "probes" (only where the data provably arrives
    before the probe executes); remaining multi-waits are split via
    event semaphores (bass_rust.generate_event_semaphores).
"""

import ml_dtypes
import numpy as np

import concourse.bass as bass
import concourse.mybir as mybir
import concourse.tile as tile
from concourse.bass import ts
from concourse.bass_utils import run_bass_kernel_spmd

H = W = 28
KH = KW = 3
CIN = H * W  # 784
HID = 128
OUT = 10
B_TOTAL = 65536
NCORES = 8
BS = B_TOTAL // NCORES  # 8192 rows per core
NB = 512  # batch columns per block (fp32 PSUM bank limit)
NBLK = BS // NB  # 16
KCH = 128
KC = 6  # full chunks (6 * 128 = 768)
KTAIL = CIN - KC * KCH  # 16
NGRP = 4  # tail row-groups / FC2 col-groups (32 partitions each)
TGC = BS // NGRP  # tail columns per group (2048)
WSPLIT = 640  # wpk head/rest split (chunks 0-4 | chunk 5 + tail + w2)
NDUMMY = 9  # pre-context HAM warm-up matmuls

HOST_DT = np.float16
X_DT = ml_dtypes.float8_e3m4
X_SCALE = 2.0  # folded into W1e on the host

# processing groups: three quads, a triple, and a final single block so
# the last FC2->act->store chain is as shallow as possible (the
# block->col-group/row-group maps are position-independent: j = bi % 4,
# q = bi // 4)
PGROUPS = [[0, 1, 2, 3], [4, 5, 6, 7], [8, 9, 10, 11], [12, 13, 14], [15]]


def _build_nc():
    f32 = mybir.dt.float32
    mdt = mybir.dt.float16
    xdt = mybir.dt.float8e3
    nc = bass.Bass()
    # x, host-pretiled per load: xa/xm entries are each one contiguous
    # [128, 6, ncols] region (features 0:768); xa = blocks 0-5, 14, 15,
    # xm = block pairs (6,7) (8,9) (10,11) (12,13)
    xa = nc.dram_tensor("xa", [8, KCH, KC, NB], xdt, kind="ExternalInput")
    xm = nc.dram_tensor("xm", [4, KCH, KC, 2 * NB], xdt, kind="ExternalInput")
    # x contraction tail (features 768:784) packed into 4 row-groups:
    # partition 32g+j = tail feature j of blocks 4g..4g+3
    xtl = nc.dram_tensor("xtl", [KCH, TGC], xdt, kind="ExternalInput")
    # all fp16 weights packed into one tensor, loaded in two pieces:
    # cols 0:768 = w1e chunks [k, c, m]; rows 32g:32g+16 of cols 768:896
    # = the 16-row w1e tail (replicated per row-group g); cols 896:906 =
    # w2t; col 906 zero (FC2 group opener)
    # first-gate transfer: block 0's first 3 chunks (1536 fp8 bytes)
    # + the weight head (WSPLIT fp16 = 1280 bytes) in ONE region, so the
    # first matmul waits on a single DMA completion instead of the max
    # of two jittery arrivals on different rings
    wx0 = nc.dram_tensor("wx0", [KCH, 1536 + 2 * WSPLIT], xdt, kind="ExternalInput")
    wpkb = nc.dram_tensor("wpkb", [KCH, 908 - WSPLIT], mdt, kind="ExternalInput")
    # biases in one f32 tensor: col 0 = b1, col 1 rows 32j:32j+10 = b2
    # (replicated per FC2 col-group j)
    bd = nc.dram_tensor("bd", [HID, 2], f32, kind="ExternalInput")
    # output, fp16, col-group packed: yt[32*(bi%4)+r, (bi//4)*512+c] =
    # y[bi*512+c, r]
    yt = nc.dram_tensor("yt", [KCH, NGRP * NB], mdt, kind="ExternalOutput")

    from contextlib import ExitStack

    es = ExitStack()
    # Pre-TileContext HAM warm-up (see module docstring). They read
    # uninitialized manual SBUF (values irrelevant) and write a scratch
    # PSUM bank nobody reads.
    wsb = es.enter_context(nc.sbuf_tensor([KCH, NB], mdt))
    wps = es.enter_context(nc.psum_tensor([KCH, NB], f32))

    for _ in range(NDUMMY):
        nc.tensor.matmul(wps[:], wsb[:, 0:HID], wsb[:], start=True, stop=True)

    with tile.TileContext(nc) as tc:
        with (
            tc.tile_pool(name="consts", bufs=1) as consts,
            tc.tile_pool(name="xin", bufs=1) as xin,
            tc.tile_pool(name="hpool", bufs=NBLK) as hpool,
            tc.tile_pool(name="ps1", bufs=5, space="PSUM") as ps1p,
            tc.tile_pool(name="ps2", bufs=2, space="PSUM") as ps2p,
        ):
            xts = {}  # block -> (tile, col offset)

            def xload(engine, tag, srcap, shape):
                t = xin.tile(shape, xdt, tag=tag, bufs=1, name=tag)
                engine.dma_start(t[:], srcap)
                return t

            # All x on the SP ring in strict consumption order;
            # the combined first-gate piece goes first.
            wx0_t = xin.tile(
                [KCH, 1536 + 2 * WSPLIT], xdt, tag="wx0", bufs=1, name="wx0"
            )
            nc.sync.dma_start(wx0_t[:], wx0[:])
            x0a = wx0_t[:, 0:1536].rearrange("k (c m) -> k c m", c=3)
            wpka_t = wx0_t[:, 1536:].bitcast(mdt)
            x0b = xload(nc.sync, "x0b", xa[0][:, 3:6, :], [KCH, 3, NB])
            xts[1] = (xload(nc.sync, "x1", xa[1][:], [KCH, KC, NB]), 0)
            x_tl = consts.tile([KCH, TGC], xdt)
            nc.sync.dma_start(x_tl[:], xtl[:])
            for bi in range(2, 6):
                xts[bi] = (xload(nc.sync, f"x{bi}", xa[bi][:], [KCH, KC, NB]), 0)
            for i in range(4):
                t = xload(nc.sync, f"xm{i}", xm[i][:], [KCH, KC, 2 * NB])
                xts[6 + 2 * i], xts[7 + 2 * i] = (t, 0), (t, NB)
            xts[14] = (xload(nc.sync, "x14", xa[6][:], [KCH, KC, NB]), 0)
            xts[15] = (xload(nc.sync, "x15", xa[7][:], [KCH, KC, NB]), 0)

            # ACT ring: only the small weight/bias transfers.
            wpkb_t = consts.tile([KCH, 908 - WSPLIT], mdt)
            nc.scalar.dma_start(wpkb_t[:], wpkb[:])
            bd_t = consts.tile([HID, 2], f32)
            nc.scalar.dma_start(bd_t[:], bd[:])
            b1_t = bd_t[:, 0:1]

            def w1_chunk(c):
                if c < 5:
                    return wpka_t[:, ts(c, KCH)]
                return wpkb_t[:, ts(c - 5, KCH)]

            wtail = wpkb_t[:, 128:256]  # [128, 128]: tail rows per group
            w2_t = wpkb_t[:, 256:266]
            wzero = wpkb_t[:, 266:267]  # zero column (FC2 opener)

            # fp16 output accumulator [128, 2048]
            o_all = consts.tile([KCH, NGRP * NB], mdt)

            # Tiny dummy bf16 ldweights "probes" absorb cross-engine
            # waits into the PE's in-order stream ahead of each matmul
            # group (walrus: one sync wait per instruction; the loaded
            # garbage weight is irrelevant, real matmuls self-load).
            def probe(ap, cast=True):
                ap = ap[0:1, 0:1]
                if cast:
                    ap = ap.bitcast(mybir.dt.bfloat16)
                nc.tensor.ldweights(ap)

            # Pre-touch the bias tile on the engines that consume it.
            b1_probe = consts.tile([1, 1], f32)
            nc.vector.tensor_copy(b1_probe[:], b1_t[0:1, 0:1])
            b2_probe = consts.tile([1, 1], f32)
            nc.scalar.copy(b2_probe[:], bd_t[0:1, 1:2])

            # Only pre-touch the weight head on the PE: the first real
            # matmul gates on it anyway.
            probe(wpka_t[:, 0:1])

            hs = [None] * NBLK

            def filler(n=1):
                """Keep the PE (and its HAM activity window) busy
                across an x-arrival stall: matmuls on already-resident
                operands into the scratch bank. Nearly free when the
                stall is real (the DMA pacer is unaffected); ~216 ns
                each when it isn't."""
                for _ in range(n):
                    nc.tensor.matmul(
                        wps[:], wpka_t[:, 0:HID], x0a[:, 0, :],
                        start=True, stop=True,
                    )

            def fc1_block(bi, ps):
                """6 accumulating FC1 matmuls for one full block."""
                if bi == 0:
                    probe(x0a[:, 0, 0:1], cast=False)
                    for c in range(3):
                        nc.tensor.matmul(
                            ps[:], w1_chunk(c), x0a[:, c, :],
                            start=(c == 0), stop=False,
                        )
                    filler(2)
                    probe(x0b[:, 0, 0:1], cast=False)
                    for c in range(3, KC):
                        if c == 5:
                            probe(wpkb_t[:, 0:1])
                        nc.tensor.matmul(
                            ps[:], w1_chunk(c), x0b[:, c - 3, :],
                            start=False, stop=False,
                        )
                else:
                    x_t, off = xts[bi]
                    probe(x_t[:, 0, off : off + 1], cast=False)
                    for c in range(KC):
                        nc.tensor.matmul(
                            ps[:], w1_chunk(c), x_t[:, c, off : off + NB],
                            start=(c == 0), stop=False,
                        )

            def tail_mm(bi, ps, c0, ncols):
                j, q = bi % NGRP, bi // NGRP
                nc.tensor.matmul(
                    ps[:, 0:ncols],
                    wtail[32 * j : 32 * j + KTAIL, :],
                    x_tl[32 * j : 32 * j + KTAIL, q * NB + c0 : q * NB + c0 + ncols],
                    start=False,
                    stop=True,
                    tile_position=(32 * j, 0),
                )

            def fc2_batch(blocks, evac_vector=False):
                """FC2 for a group of blocks (software-pipelined one
                group late). A tiny N=1 matmul opens the accumulation
                group (clears has_written + pending-zero for the bank);
                the FC2 matmuls then target col-groups j=bi%4 of that
                one bank (disjoint partitions, start=False = overwrite-
                where-unwritten) and run concurrently in the array; one
                bias-add over the written partition range evacuates the
                group."""
                q = blocks[0] // NGRP
                ps2 = ps2p.tile([KCH, NB], f32, tag="ps2", bufs=2)
                nc.tensor.matmul(
                    ps2[:, 0:1], wtail, wzero, start=True, stop=False
                )
                for n, bi in enumerate(blocks):
                    j = bi % NGRP
                    nc.tensor.matmul(
                        ps2[32 * j : 32 * j + OUT, :],
                        w2_t[:],
                        hs[bi][:],
                        start=False,
                        stop=(n == len(blocks) - 1),
                        tile_position=(0, 32 * j),
                    )
                j0, j1 = blocks[0] % NGRP, blocks[-1] % NGRP
                lo, hi = 32 * j0, 32 * j1 + 32
                if evac_vector:
                    nc.vector.tensor_scalar(
                        o_all[lo:hi, ts(q, NB)],
                        ps2[lo:hi, :],
                        bd_t[lo:hi, 1:2],
                        None,
                        mybir.AluOpType.add,
                    )
                else:
                    nc.scalar.activation(
                        o_all[lo:hi, ts(q, NB)],
                        ps2[lo:hi, :],
                        mybir.ActivationFunctionType.Identity,
                        bias=bd_t[lo:hi, 1:2],
                    )

            def relu(bi, ps, h_ap, ncols, on_vector):
                """h = max(ps + b1, 0), PSUM fp32 -> SBUF fp16."""
                if on_vector:
                    nc.vector.tensor_scalar(
                        h_ap,
                        ps[:, 0:ncols],
                        b1_t[:],
                        0.0,
                        mybir.AluOpType.add,
                        mybir.AluOpType.max,
                    )
                else:
                    nc.scalar.activation(
                        h_ap,
                        ps[:, 0:ncols],
                        mybir.ActivationFunctionType.Relu,
                        bias=b1_t[:],
                    )

            # ---- main pipeline: groups 0..3 ----
            for qi in range(4):
                blocks = PGROUPS[qi]
                pss = []
                for bi in blocks:
                    if bi == 1:
                        filler(3)
                    elif bi == 2:
                        filler(1)
                    ps = ps1p.tile([HID, NB], f32, tag="ps")
                    fc1_block(bi, ps)
                    pss.append(ps)
                for n, bi in enumerate(blocks):
                    tail_mm(bi, pss[n], 0, NB)
                for n, bi in enumerate(blocks):
                    h = hpool.tile([HID, NB], mdt, tag="h", name=f"h_{bi}")
                    relu(bi, pss[n], h[:], NB, on_vector=bi not in (11, 13))
                    hs[bi] = h
                if qi >= 1:
                    fc2_batch(PGROUPS[qi - 1])
                if qi == 3:
                    # cols 0:1536 (q=0,1,2) complete after fc2_batch
                    # (PGROUPS[2]) just above: bulk store overlaps the
                    # rest of the compute (issued from the idle Sync
                    # queue so the ACT engine stays clear)
                    nc.sync.dma_start(yt[:, 0 : 3 * NB], o_all[:, 0 : 3 * NB])

            # ---- epilogue: block 15, two half-banks for a parallel
            # relu and the shallowest possible final chain ----
            psA = ps1p.tile([HID, NB], f32, tag="ps")
            psB = ps1p.tile([HID, NB], f32, tag="ps")
            x_t, off = xts[15]
            probe(x_t[:, 0, off : off + 1], cast=False)
            for c in range(KC):
                wc = w1_chunk(c)
                nc.tensor.matmul(
                    psA[:, 0:256], wc, x_t[:, c, off : off + 256],
                    start=(c == 0), stop=False,
                )
                nc.tensor.matmul(
                    psB[:, 0:256], wc, x_t[:, c, off + 256 : off + NB],
                    start=(c == 0), stop=False,
                )
            # block 15 tails + relus fire first (relu15b is the only
            # pre-final-act ACT work); FC2 of [12,13,14] fills the PE
            # while they resolve, its bias-add runs on DVE and its
            # store issues from the idle Sync queue - the ACT engine
            # goes straight from relu15b to the final bias-adds + store.
            tail_mm(15, psA, 0, 256)
            tail_mm(15, psB, 256, 256)
            h15 = hpool.tile([HID, NB], mdt, tag="h", name="h_15")
            relu(15, psA, h15[:, 0:256], 256, on_vector=True)
            relu(15, psB, h15[:, 256:NB], 256, on_vector=False)
            hs[15] = h15
            fc2_batch(PGROUPS[3], evac_vector=True)
            nc.sync.dma_start(yt[0:96, 3 * NB :], o_all[0:96, 3 * NB :])
            # final FC2 as two independent half-bank chains: each half
            # gates only on its own relu half, and the two [32, 256]
            # bias-adds are cheaper than one [32, 512] on the critical
            # path (different PSUM banks, so no read-write collision).
            # Half b uses the pre-context scratch bank (free since the
            # dummies) so neither half waits on ps2-pool recycling,
            # which would chain it behind [12,13,14]'s late evacuation.
            for half in range(2):
                c0 = 256 * half
                if half == 0:
                    ps2h = ps2p.tile([KCH, NB], f32, tag="ps2", bufs=2)
                    ps2 = ps2h[:]
                else:
                    ps2 = wps[:]
                nc.tensor.matmul(
                    ps2[96:106, 0:256],
                    w2_t[:],
                    h15[:, c0 : c0 + 256],
                    start=True,
                    stop=True,
                    tile_position=(0, 96),
                )
                nc.scalar.activation(
                    o_all[96:128, 3 * NB + c0 : 3 * NB + c0 + 256],
                    ps2[96:128, 0:256],
                    mybir.ActivationFunctionType.Identity,
                    bias=bd_t[96:128, 1:2],
                )

            # Final store (32 KB) from the ACT sequencer: program order
            # after the acts means it needs no cross-engine waits at all.
            nc.scalar.dma_start(yt[96:128, 3 * NB :], o_all[96:128, 3 * NB :])

    # This walrus build allows one sync-wait per instruction; Tile emits
    # multi-waits in a few places. Split them into event-semaphore
    # chains, same as bacc.compile() does.
    import bass_rust

    bass_rust.generate_event_semaphores(nc)
    es.close()
    return nc


def _fuse_conv_fc1(conv_w, w1):
    """W1e = w1 @ C where C is the 3x3 valid-conv operator [676, 784]."""
    cw = np.asarray(conv_w, np.float64).reshape(KH, KW)
    w1_r = np.asarray(w1, np.float64).reshape(HID, H - KH + 1, W - KW + 1)
    w1e = np.zeros((HID, H, W), np.float64)
    for a in range(KH):
        for b in range(KW):
            w1e[:, a : a + H - KH + 1, b : b + W - KW + 1] += w1_r * cw[a, b]
    return w1e.reshape(HID, CIN).astype(np.float32)


def _tile_cols(x_shard, cs, ncols):
    """[128, 6, ncols] contiguous device layout for columns cs:cs+ncols."""
    return (
        x_shard[cs : cs + ncols, : KC * KCH]
        .reshape(ncols, KC, KCH)
        .transpose(2, 1, 0)
        .astype(X_DT)
    )


def _core_x(x_shard):
    """Pre-tile one core's x rows [BS, 784] into the device layout.
    x arrives pre-scaled by X_SCALE."""
    xa = np.stack(
        [_tile_cols(x_shard, bi * NB, NB) for bi in range(6)]
        + [
            _tile_cols(x_shard, BS - 2 * NB, NB),
            _tile_cols(x_shard, BS - NB, NB),
        ]
    )
    xm = np.stack(
        [_tile_cols(x_shard, 6 * NB + 2 * NB * i, 2 * NB) for i in range(4)]
    )
    xtl = np.zeros((KCH, TGC), X_DT)
    tail = x_shard[:, KC * KCH :].astype(X_DT)  # [BS, 16]
    for bi in range(NBLK):
        q, j = divmod(bi, NGRP)
        xtl[32 * j : 32 * j + KTAIL, q * NB : (q + 1) * NB] = tail[
            bi * NB : (bi + 1) * NB
        ].T
    return (
        np.ascontiguousarray(xa),
        np.ascontiguousarray(xm),
        np.ascontiguousarray(xtl),
    )


def _host_weights(conv_w, w1, b1, w2, b2):
    """Pack all fp16 weights into wpk [128, 908] and biases into bd."""
    # 1/X_SCALE folds into W1e (exact in fp16: pure exponent shift)
    w1e_t = (_fuse_conv_fc1(conv_w, w1).T / X_SCALE).astype(HOST_DT)  # [784, 128]
    w2t = np.asarray(w2, np.float32).T.astype(HOST_DT)  # [128, 10]
    wpk = np.zeros((KCH, 908), HOST_DT)
    wpk[:, 0:768] = (
        w1e_t[0 : KC * KCH].reshape(KC, KCH, HID).transpose(1, 0, 2).reshape(KCH, -1)
    )
    for g in range(NGRP):
        wpk[32 * g : 32 * g + KTAIL, 768:896] = w1e_t[KC * KCH :]
    wpk[:, 896:906] = w2t
    bd = np.zeros((HID, 2), np.float32)
    bd[:, 0] = np.asarray(b1, np.float32)
    for j in range(NGRP):
        bd[32 * j : 32 * j + OUT, 1] = np.asarray(b2, np.float32)
    return (
        np.ascontiguousarray(wpk[:, :WSPLIT]),
        np.ascontiguousarray(wpk[:, WSPLIT:]),
        np.ascontiguousarray(bd),
    )


def _run(x, conv_w, w1, b1, w2, b2, trace=False):
    x = np.asarray(x, np.float32) * np.float32(X_SCALE)
    wpka, wpkb, bd = _host_weights(conv_w, w1, b1, w2, b2)
    wpka_bytes = np.ascontiguousarray(wpka).view(np.uint8)  # [128, 1024+256]

    nc = _build_nc()
    in_maps = []
    for c in range(NCORES):
        xa, xm, xtl = _core_x(x[c * BS : (c + 1) * BS])
        wx0 = np.ascontiguousarray(
            np.concatenate(
                [xa[0, :, 0:3, :].reshape(KCH, 3 * NB).view(np.uint8), wpka_bytes],
                axis=1,
            ).view(X_DT)
        )
        in_maps.append(
            {"xa": xa, "xm": xm, "xtl": xtl, "wx0": wx0, "wpkb": wpkb, "bd": bd}
        )
    res = run_bass_kernel_spmd(nc, in_maps, list(range(NCORES)), trace=trace)

    y = np.empty((B_TOTAL, OUT), np.float32)
    for c, r in enumerate(res.results):
        # yt[32j+r, 512q+cc] = y[(4q+j)*512+cc, r]
        ytc = np.asarray(r["yt"], np.float32).reshape(NGRP, 32, NGRP, NB)[:, :OUT]
        y[c * BS : (c + 1) * BS] = ytc.transpose(2, 0, 3, 1).reshape(BS, OUT)
    return y, res


def kernel(x, conv_w, w1, b1, w2, b2):
    y, _ = _run(x, conv_w, w1, b1, w2, b2)
    return y


# revision 32
# speedup vs baseline: 1.0259x; 1.0038x over previous
"""Trainium2 Bass kernel for nn_DigitConvolutionalModel (dense_cnn).

Model: y = relu(conv3x3(x) @ w1.T + b1) @ w2.T + b2, x: [65536, 784] f32.

Strategy:
  * Conv3x3 and FC1 fuse on the host into one effective weight
    W1e = w1 @ C with shape [128, 784] (C is the sparse conv operator),
    so the device runs a pure GEMM pipeline:
    y = relu(x @ W1e.T + b1) @ w2.T + b2.
  * Pure data parallel over 8 NeuronCores: each core gets 8192 rows of
    x. No collectives; each core produces its own output shard.
  * x travels as fp8e3 (e3m4), scaled by 2 on the host with the inverse
    folded into W1e (fp16) - mixed-dtype PE operands, fp32 PSUM
    accumulation; x quantization costs ~1.26e-2 rel_fro (gate 2e-2).
  * The kernel is PE-roofline bound when warm (96 N=512 fp16-rate
    matmuls ~20.7 us) with the x stream (~6.4 MB fp8) just underneath;
    the optimization battle is the edges:
    - All x on the SP HWDGE ring in strict consumption order. The SDMA
      engines round-robin across every outstanding transfer, so any
      concurrent stream dilutes the first loads, which gate warm-up:
      the first transfer fuses block 0's first 3 chunks WITH the fp16
      weight head into one 352 KB region (bitcast views on the SBUF
      tile), so the first matmul gates on a single DMA completion;
      block 0's rest follows as a 3-chunk piece, and the middle blocks
      ride 1 MB pair-loads whose 6 KB descriptors run near the full
      HBM rate. The ACT ring carries only the small remaining
      weight/bias transfers.
    - 9 dummy pre-context matmuls bridge the PE from the framework
      preamble (~7 us) to the first x piece's consumability (~11 us:
      first HWDGE issue slot + descriptor generation + transfer +
      completion receipt) with NO idle gap - the HAM activity window
      only unthrottles the PE clock (1.2 -> 2.4 GHz) after ~3.4 us of
      CONTINUOUS busy, and any gap restarts it.
    - The packed weights split at column 640 (chunks 0-4 | chunk 5 +
      tail + w2) so block 0's last chunk is the only matmul gated on
      the second piece, which lands well before it.
    - Filler matmuls (already-resident operands -> scratch bank)
      pad the two early x-arrival stalls (block 0's second piece,
      block 1) so the PE never idles: nearly free when the stall is
      real, and they keep the HAM window accumulating.
    - The exit chain is minimized: groups [0-3],[4-7],[8-11],
      [12,13,14],[15]; block 15's FC1 runs as two half-banks (N=256)
      so its bias+relu runs on BOTH the vector and scalar engines in
      parallel (different PSUM banks); the final FC2 runs as two
      single-matmul half-bank chains (start=True needs no opener when
      nothing writes the bank concurrently; one half reuses the
      scratch bank to dodge pool recycling), each gating only on its
      own relu half, with two [32, 256] bias-adds and one 32 KB final
      store. All bulk stores issue from the otherwise-idle Sync queue
      so the ACT engine is clear for the final chain; fc2 of
      [12,13,14] evacuates through the vector engine for the same
      reason. The grouped FC2s keep their N=1 opener matmul: a
      start=True on one of several concurrent col-group matmuls races
      the bank clear against the others' writes (HW-verified wrong
      results).
  * The 16-feature contraction tail (features 768:784, whole batch) is
    packed [128, 2048] across 4 row-groups of 32 partitions; w1e's
    tail rows are replicated at partition offsets 0/32/64/96 so each
    block's tail matmul reads its group via tile_position (row-group =
    bi % 4). A group's tail matmuls issue back-to-back on distinct
    row-groups and run concurrently in the PE array.
  * Bias+ReLU (PSUM -> SBUF fp16) alternates between the vector and
    scalar engines so a group's relus don't serialize on one engine.
  * FC2 runs software-pipelined one group late: a tiny N=1 matmul with
    a zero moving operand opens the accumulation group, then the
    group's [10, 512] FC2 matmuls target col-groups j=bi%4 of that ONE
    bank (disjoint partitions, start=False) and run concurrently in
    the array; one bias-add per group evacuates the bank into a
    [128, 2048] fp16 output accumulator.
  * Cross-engine waits are absorbed into the PE stream with tiny dummy
    bf16 ldweights "probes" (only where the data provably arrives
    before the probe executes); remaining multi-waits are split via
    event semaphores (bass_rust.generate_event_semaphores).
"""

import ml_dtypes
import numpy as np

import concourse.bass as bass
import concourse.mybir as mybir
import concourse.tile as tile
from concourse.bass import ts
from concourse.bass_utils import run_bass_kernel_spmd

H = W = 28
KH = KW = 3
CIN = H * W  # 784
HID = 128
OUT = 10
B_TOTAL = 65536
NCORES = 8
BS = B_TOTAL // NCORES  # 8192 rows per core
NB = 512  # batch columns per block (fp32 PSUM bank limit)
NBLK = BS // NB  # 16
KCH = 128
KC = 6  # full chunks (6 * 128 = 768)
KTAIL = CIN - KC * KCH  # 16
NGRP = 4  # tail row-groups / FC2 col-groups (32 partitions each)
TGC = BS // NGRP  # tail columns per group (2048)
WSPLIT = 640  # wpk head/rest split (chunks 0-4 | chunk 5 + tail + w2)
NDUMMY = 9  # pre-context HAM warm-up matmuls

HOST_DT = np.float16
X_DT = ml_dtypes.float8_e3m4
X_SCALE = 2.0  # folded into W1e on the host

# processing groups: three quads, a triple, and a final single block so
# the last FC2->act->store chain is as shallow as possible (the
# block->col-group/row-group maps are position-independent: j = bi % 4,
# q = bi // 4)
PGROUPS = [[0, 1, 2, 3], [4, 5, 6, 7], [8, 9, 10, 11], [12, 13, 14], [15]]


def _build_nc():
    f32 = mybir.dt.float32
    mdt = mybir.dt.float16
    xdt = mybir.dt.float8e3
    nc = bass.Bass()
    # x, host-pretiled per load: xa/xm entries are each one contiguous
    # [128, 6, ncols] region (features 0:768); xa = blocks 0-5, 14, 15,
    # xm = block pairs (6,7) (8,9) (10,11) (12,13)
    xa = nc.dram_tensor("xa", [8, KCH, KC, NB], xdt, kind="ExternalInput")
    xm = nc.dram_tensor("xm", [4, KCH, KC, 2 * NB], xdt, kind="ExternalInput")
    # x contraction tail (features 768:784, 4 row-groups: partition
    # 32g+j = tail feature j of blocks 4g..4g+3) fused with the f32
    # bias tensor (col 0 = b1, col 1 rows 32j:32j+10 = b2): one ACT
    # transfer, one completion
    xbd = nc.dram_tensor("xbd", [KCH, TGC + 8], xdt, kind="ExternalInput")
    # all fp16 weights packed into one tensor, loaded in two pieces:
    # cols 0:768 = w1e chunks [k, c, m]; rows 32g:32g+16 of cols 768:896
    # = the 16-row w1e tail (replicated per row-group g); cols 896:906 =
    # w2t; col 906 zero (FC2 group opener)
    # first-gate transfer: block 0's first 3 chunks (1536 fp8 bytes)
    # + the weight head (WSPLIT fp16 = 1280 bytes) in ONE region, so the
    # first matmul waits on a single DMA completion instead of the max
    # of two jittery arrivals on different rings
    wx0 = nc.dram_tensor("wx0", [KCH, 1536 + 2 * WSPLIT], xdt, kind="ExternalInput")
    # block 0's last 3 chunks fused with the weight rest (chunk 5 +
    # tail + w2 + zero col, 268 fp16 = 536 bytes): one SP transfer
    wx0b = nc.dram_tensor(
        "wx0b", [KCH, 1536 + 2 * (908 - WSPLIT)], xdt, kind="ExternalInput"
    )
    # output, fp16, col-group packed: yt[32*(bi%4)+r, (bi//4)*512+c] =
    # y[bi*512+c, r]
    yt = nc.dram_tensor("yt", [KCH, NGRP * NB], mdt, kind="ExternalOutput")

    from contextlib import ExitStack

    es = ExitStack()
    # Pre-TileContext HAM warm-up (see module docstring). They read
    # uninitialized manual SBUF (values irrelevant) and write a scratch
    # PSUM bank nobody reads.
    wsb = es.enter_context(nc.sbuf_tensor([KCH, NB], mdt))
    wps = es.enter_context(nc.psum_tensor([KCH, NB], f32))

    for _ in range(NDUMMY):
        nc.tensor.matmul(wps[:], wsb[:, 0:HID], wsb[:], start=True, stop=True)

    with tile.TileContext(nc) as tc:
        with (
            tc.tile_pool(name="consts", bufs=1) as consts,
            tc.tile_pool(name="xin", bufs=1) as xin,
            tc.tile_pool(name="hpool", bufs=NBLK) as hpool,
            tc.tile_pool(name="ps1", bufs=5, space="PSUM") as ps1p,
            tc.tile_pool(name="ps2", bufs=2, space="PSUM") as ps2p,
        ):
            xts = {}  # block -> (tile, col offset)

            def xload(engine, tag, srcap, shape):
                t = xin.tile(shape, xdt, tag=tag, bufs=1, name=tag)
                engine.dma_start(t[:], srcap)
                return t

            # All x on the SP ring in strict consumption order;
            # the combined first-gate piece goes first.
            wx0_t = xin.tile(
                [KCH, 1536 + 2 * WSPLIT], xdt, tag="wx0", bufs=1, name="wx0"
            )
            nc.sync.dma_start(wx0_t[:], wx0[:])
            x0a = wx0_t[:, 0:1536].rearrange("k (c m) -> k c m", c=3)
            wpka_t = wx0_t[:, 1536:].bitcast(mdt)
            wx0b_t = xin.tile(
                [KCH, 1536 + 2 * (908 - WSPLIT)], xdt, tag="wx0b", bufs=1,
                name="wx0b",
            )
            nc.sync.dma_start(wx0b_t[:], wx0b[:])
            x0b = wx0b_t[:, 0:1536].rearrange("k (c m) -> k c m", c=3)
            wpkb_t = wx0b_t[:, 1536:].bitcast(mdt)
            xts[1] = (xload(nc.sync, "x1", xa[1][:], [KCH, KC, NB]), 0)
            for bi in range(2, 6):
                xts[bi] = (xload(nc.sync, f"x{bi}", xa[bi][:], [KCH, KC, NB]), 0)
            for i in range(4):
                t = xload(nc.sync, f"xm{i}", xm[i][:], [KCH, KC, 2 * NB])
                xts[6 + 2 * i], xts[7 + 2 * i] = (t, 0), (t, NB)
            xts[14] = (xload(nc.sync, "x14", xa[6][:], [KCH, KC, NB]), 0)
            xts[15] = (xload(nc.sync, "x15", xa[7][:], [KCH, KC, NB]), 0)

            # ACT ring: one fused tail+bias transfer.
            xbd_t = consts.tile([KCH, TGC + 8], xdt)
            nc.scalar.dma_start(xbd_t[:], xbd[:])
            x_tl = xbd_t[:, 0:TGC]
            bd_t = xbd_t[:, TGC:].bitcast(f32)
            b1_t = bd_t[:, 0:1]

            def w1_chunk(c):
                if c < 5:
                    return wpka_t[:, ts(c, KCH)]
                return wpkb_t[:, ts(c - 5, KCH)]

            wtail = wpkb_t[:, 128:256]  # [128, 128]: tail rows per group
            w2_t = wpkb_t[:, 256:266]
            wzero = wpkb_t[:, 266:267]  # zero column (FC2 opener)

            # fp16 output accumulator [128, 2048]
            o_all = consts.tile([KCH, NGRP * NB], mdt)

            # Tiny dummy bf16 ldweights "probes" absorb cross-engine
            # waits into the PE's in-order stream ahead of each matmul
            # group (walrus: one sync wait per instruction; the loaded
            # garbage weight is irrelevant, real matmuls self-load).
            def probe(ap, cast=True):
                ap = ap[0:1, 0:1]
                if cast:
                    ap = ap.bitcast(mybir.dt.bfloat16)
                nc.tensor.ldweights(ap)

            # Pre-touch the bias tile on the engines that consume it.
            b1_probe = consts.tile([1, 1], f32)
            nc.vector.tensor_copy(b1_probe[:], b1_t[0:1, 0:1])
            b2_probe = consts.tile([1, 1], f32)
            nc.scalar.copy(b2_probe[:], bd_t[0:1, 1:2])

            # Only pre-touch the weight head on the PE: the first real
            # matmul gates on it anyway.
            probe(wpka_t[:, 0:1])

            hs = [None] * NBLK

            def filler(n=1):
                """Keep the PE (and its HAM activity window) busy
                across an x-arrival stall: matmuls on already-resident
                operands into the scratch bank. Nearly free when the
                stall is real (the DMA pacer is unaffected); ~216 ns
                each when it isn't."""
                for _ in range(n):
                    nc.tensor.matmul(
                        wps[:], wpka_t[:, 0:HID], x0a[:, 0, :],
                        start=True, stop=True,
                    )

            def fc1_block(bi, ps):
                """6 accumulating FC1 matmuls for one full block."""
                if bi == 0:
                    probe(x0a[:, 0, 0:1], cast=False)
                    for c in range(3):
                        nc.tensor.matmul(
                            ps[:], w1_chunk(c), x0a[:, c, :],
                            start=(c == 0), stop=False,
                        )
                    filler(2)
                    probe(x0b[:, 0, 0:1], cast=False)
                    for c in range(3, KC):
                        if c == 5:
                            probe(wpkb_t[:, 0:1])
                        nc.tensor.matmul(
                            ps[:], w1_chunk(c), x0b[:, c - 3, :],
                            start=False, stop=False,
                        )
                else:
                    x_t, off = xts[bi]
                    probe(x_t[:, 0, off : off + 1], cast=False)
                    for c in range(KC):
                        nc.tensor.matmul(
                            ps[:], w1_chunk(c), x_t[:, c, off : off + NB],
                            start=(c == 0), stop=False,
                        )

            def tail_mm(bi, ps, c0, ncols):
                j, q = bi % NGRP, bi // NGRP
                nc.tensor.matmul(
                    ps[:, 0:ncols],
                    wtail[32 * j : 32 * j + KTAIL, :],
                    x_tl[32 * j : 32 * j + KTAIL, q * NB + c0 : q * NB + c0 + ncols],
                    start=False,
                    stop=True,
                    tile_position=(32 * j, 0),
                )

            def fc2_batch(blocks, evac_vector=False):
                """FC2 for a group of blocks (software-pipelined one
                group late). A tiny N=1 matmul opens the accumulation
                group (clears has_written + pending-zero for the bank);
                the FC2 matmuls then target col-groups j=bi%4 of that
                one bank (disjoint partitions, start=False = overwrite-
                where-unwritten) and run concurrently in the array; one
                bias-add over the written partition range evacuates the
                group."""
                q = blocks[0] // NGRP
                ps2 = ps2p.tile([KCH, NB], f32, tag="ps2", bufs=2)
                nc.tensor.matmul(
                    ps2[:, 0:1], wtail, wzero, start=True, stop=False
                )
                for n, bi in enumerate(blocks):
                    j = bi % NGRP
                    nc.tensor.matmul(
                        ps2[32 * j : 32 * j + OUT, :],
                        w2_t[:],
                        hs[bi][:],
                        start=False,
                        stop=(n == len(blocks) - 1),
                        tile_position=(0, 32 * j),
                    )
                j0, j1 = blocks[0] % NGRP, blocks[-1] % NGRP
                lo, hi = 32 * j0, 32 * j1 + 32
                if evac_vector:
                    nc.vector.tensor_scalar(
                        o_all[lo:hi, ts(q, NB)],
                        ps2[lo:hi, :],
                        bd_t[lo:hi, 1:2],
                        None,
                        mybir.AluOpType.add,
                    )
                else:
                    nc.scalar.activation(
                        o_all[lo:hi, ts(q, NB)],
                        ps2[lo:hi, :],
                        mybir.ActivationFunctionType.Identity,
                        bias=bd_t[lo:hi, 1:2],
                    )

            def relu(bi, ps, h_ap, ncols, on_vector):
                """h = max(ps + b1, 0), PSUM fp32 -> SBUF fp16."""
                if on_vector:
                    nc.vector.tensor_scalar(
                        h_ap,
                        ps[:, 0:ncols],
                        b1_t[:],
                        0.0,
                        mybir.AluOpType.add,
                        mybir.AluOpType.max,
                    )
                else:
                    nc.scalar.activation(
                        h_ap,
                        ps[:, 0:ncols],
                        mybir.ActivationFunctionType.Relu,
                        bias=b1_t[:],
                    )

            # ---- main pipeline: groups 0..3 ----
            for qi in range(4):
                blocks = PGROUPS[qi]
                pss = []
                for bi in blocks:
                    if bi == 1:
                        filler(3)
                    elif bi == 2:
                        filler(1)
                    ps = ps1p.tile([HID, NB], f32, tag="ps")
                    fc1_block(bi, ps)
                    pss.append(ps)
                for n, bi in enumerate(blocks):
                    tail_mm(bi, pss[n], 0, NB)
                for n, bi in enumerate(blocks):
                    h = hpool.tile([HID, NB], mdt, tag="h", name=f"h_{bi}")
                    relu(bi, pss[n], h[:], NB, on_vector=bi not in (11, 13))
                    hs[bi] = h
                if qi >= 1:
                    fc2_batch(PGROUPS[qi - 1])
                if qi == 3:
                    # cols 0:1536 (q=0,1,2) complete after fc2_batch
                    # (PGROUPS[2]) just above: bulk store overlaps the
                    # rest of the compute (issued from the idle Sync
                    # queue so the ACT engine stays clear)
                    nc.sync.dma_start(yt[:, 0 : 3 * NB], o_all[:, 0 : 3 * NB])

            # ---- epilogue: block 15, two half-banks for a parallel
            # relu and the shallowest possible final chain ----
            psA = ps1p.tile([HID, NB], f32, tag="ps")
            psB = ps1p.tile([HID, NB], f32, tag="ps")
            x_t, off = xts[15]
            probe(x_t[:, 0, off : off + 1], cast=False)
            for c in range(KC):
                wc = w1_chunk(c)
                nc.tensor.matmul(
                    psA[:, 0:256], wc, x_t[:, c, off : off + 256],
                    start=(c == 0), stop=False,
                )
                nc.tensor.matmul(
                    psB[:, 0:256], wc, x_t[:, c, off + 256 : off + NB],
                    start=(c == 0), stop=False,
                )
            # block 15 tails + relus fire first (relu15b is the only
            # pre-final-act ACT work); FC2 of [12,13,14] fills the PE
            # while they resolve, its bias-add runs on DVE and its
            # store issues from the idle Sync queue - the ACT engine
            # goes straight from relu15b to the final bias-adds + store.
            tail_mm(15, psA, 0, 256)
            tail_mm(15, psB, 256, 256)
            h15 = hpool.tile([HID, NB], mdt, tag="h", name="h_15")
            relu(15, psA, h15[:, 0:256], 256, on_vector=True)
            relu(15, psB, h15[:, 256:NB], 256, on_vector=False)
            hs[15] = h15
            fc2_batch(PGROUPS[3], evac_vector=True)
            nc.sync.dma_start(yt[0:96, 3 * NB :], o_all[0:96, 3 * NB :])
            # final FC2 as two independent half-bank chains: each half
            # gates only on its own relu half, and the two [32, 256]
            # bias-adds are cheaper than one [32, 512] on the critical
            # path (different PSUM banks, so no read-write collision).
            # Half b uses the pre-context scratch bank (free since the
            # dummies) so neither half waits on ps2-pool recycling,
            # which would chain it behind [12,13,14]'s late evacuation.
            for half in range(2):
                c0 = 256 * half
                if half == 0:
                    ps2h = ps2p.tile([KCH, NB], f32, tag="ps2", bufs=2)
                    ps2 = ps2h[:]
                else:
                    ps2 = wps[:]
                nc.tensor.matmul(
                    ps2[96:106, 0:256],
                    w2_t[:],
                    h15[:, c0 : c0 + 256],
                    start=True,
                    stop=True,
                    tile_position=(0, 96),
                )
                nc.scalar.activation(
                    o_all[96:128, 3 * NB + c0 : 3 * NB + c0 + 256],
                    ps2[96:128, 0:256],
                    mybir.ActivationFunctionType.Identity,
                    bias=bd_t[96:128, 1:2],
                )

            # Final store (32 KB) from the ACT sequencer: program order
            # after the acts means it needs no cross-engine waits at all.
            nc.scalar.dma_start(yt[96:128, 3 * NB :], o_all[96:128, 3 * NB :])

    # This walrus build allows one sync-wait per instruction; Tile emits
    # multi-waits in a few places. Split them into event-semaphore
    # chains, same as bacc.compile() does.
    import bass_rust

    bass_rust.generate_event_semaphores(nc)
    es.close()
    return nc


def _fuse_conv_fc1(conv_w, w1):
    """W1e = w1 @ C where C is the 3x3 valid-conv operator [676, 784]."""
    cw = np.asarray(conv_w, np.float64).reshape(KH, KW)
    w1_r = np.asarray(w1, np.float64).reshape(HID, H - KH + 1, W - KW + 1)
    w1e = np.zeros((HID, H, W), np.float64)
    for a in range(KH):
        for b in range(KW):
            w1e[:, a : a + H - KH + 1, b : b + W - KW + 1] += w1_r * cw[a, b]
    return w1e.reshape(HID, CIN).astype(np.float32)


def _tile_cols(x_shard, cs, ncols):
    """[128, 6, ncols] contiguous device layout for columns cs:cs+ncols."""
    return (
        x_shard[cs : cs + ncols, : KC * KCH]
        .reshape(ncols, KC, KCH)
        .transpose(2, 1, 0)
        .astype(X_DT)
    )


def _core_x(x_shard):
    """Pre-tile one core's x rows [BS, 784] into the device layout.
    x arrives pre-scaled by X_SCALE."""
    xa = np.stack(
        [_tile_cols(x_shard, bi * NB, NB) for bi in range(6)]
        + [
            _tile_cols(x_shard, BS - 2 * NB, NB),
            _tile_cols(x_shard, BS - NB, NB),
        ]
    )
    xm = np.stack(
        [_tile_cols(x_shard, 6 * NB + 2 * NB * i, 2 * NB) for i in range(4)]
    )
    xtl = np.zeros((KCH, TGC), X_DT)
    tail = x_shard[:, KC * KCH :].astype(X_DT)  # [BS, 16]
    for bi in range(NBLK):
        q, j = divmod(bi, NGRP)
        xtl[32 * j : 32 * j + KTAIL, q * NB : (q + 1) * NB] = tail[
            bi * NB : (bi + 1) * NB
        ].T
    return (
        np.ascontiguousarray(xa),
        np.ascontiguousarray(xm),
        np.ascontiguousarray(xtl),
    )


def _host_weights(conv_w, w1, b1, w2, b2):
    """Pack all fp16 weights into wpk [128, 908] and biases into bd."""
    # 1/X_SCALE folds into W1e (exact in fp16: pure exponent shift)
    w1e_t = (_fuse_conv_fc1(conv_w, w1).T / X_SCALE).astype(HOST_DT)  # [784, 128]
    w2t = np.asarray(w2, np.float32).T.astype(HOST_DT)  # [128, 10]
    wpk = np.zeros((KCH, 908), HOST_DT)
    wpk[:, 0:768] = (
        w1e_t[0 : KC * KCH].reshape(KC, KCH, HID).transpose(1, 0, 2).reshape(KCH, -1)
    )
    for g in range(NGRP):
        wpk[32 * g : 32 * g + KTAIL, 768:896] = w1e_t[KC * KCH :]
    wpk[:, 896:906] = w2t
    bd = np.zeros((HID, 2), np.float32)
    bd[:, 0] = np.asarray(b1, np.float32)
    for j in range(NGRP):
        bd[32 * j : 32 * j + OUT, 1] = np.asarray(b2, np.float32)
    return (
        np.ascontiguousarray(wpk[:, :WSPLIT]),
        np.ascontiguousarray(wpk[:, WSPLIT:]),
        np.ascontiguousarray(bd),
    )


def _run(x, conv_w, w1, b1, w2, b2, trace=False):
    x = np.asarray(x, np.float32) * np.float32(X_SCALE)
    wpka, wpkb, bd = _host_weights(conv_w, w1, b1, w2, b2)
    wpka_bytes = np.ascontiguousarray(wpka).view(np.uint8)
    wpkb_bytes = np.ascontiguousarray(wpkb).view(np.uint8)
    bd_bytes = np.ascontiguousarray(bd).view(np.uint8)

    nc = _build_nc()
    in_maps = []
    for c in range(NCORES):
        xa, xm, xtl = _core_x(x[c * BS : (c + 1) * BS])
        wx0 = np.ascontiguousarray(
            np.concatenate(
                [xa[0, :, 0:3, :].reshape(KCH, 3 * NB).view(np.uint8), wpka_bytes],
                axis=1,
            ).view(X_DT)
        )
        wx0b = np.ascontiguousarray(
            np.concatenate(
                [xa[0, :, 3:6, :].reshape(KCH, 3 * NB).view(np.uint8), wpkb_bytes],
                axis=1,
            ).view(X_DT)
        )
        xbd = np.ascontiguousarray(
            np.concatenate([xtl.view(np.uint8), bd_bytes], axis=1).view(X_DT)
        )
        in_maps.append({"xa": xa, "xm": xm, "wx0": wx0, "wx0b": wx0b, "xbd": xbd})
    res = run_bass_kernel_spmd(nc, in_maps, list(range(NCORES)), trace=trace)

    y = np.empty((B_TOTAL, OUT), np.float32)
    for c, r in enumerate(res.results):
        # yt[32j+r, 512q+cc] = y[(4q+j)*512+cc, r]
        ytc = np.asarray(r["yt"], np.float32).reshape(NGRP, 32, NGRP, NB)[:, :OUT]
        y[c * BS : (c + 1) * BS] = ytc.transpose(2, 0, 3, 1).reshape(BS, OUT)
    return y, res


def kernel(x, conv_w, w1, b1, w2, b2):
    y, _ = _run(x, conv_w, w1, b1, w2, b2)
    return y
